# revision 20
# baseline (speedup 1.0000x reference)
"""GCNConv (multi-edgeset) Trainium2 kernel.

Strategy (8 NeuronCores, SPMD, sharded by destination node / col ranges):
  - Host: append self-loops, compute per-edge scale s = ew * rsqrt(deg_row) *
    rsqrt(deg_col), bucket edges by (core, block) where core owns 1250 dest
    nodes split into 10 blocks of 125; pad every (core, block) bucket to the
    same tile count T_blk (128 edges per tile).
  - Device, per 1024-edge chunk (8 tiles of 128 edges; all matmul operands bf16,
    fp32 PSUM accumulation):
      x[row] rows fetched by dma_gather (SWDGE custom op), 2048 edges per call,
        round-robin over 4 SWDGE queues so Q7 descriptor-gen overlaps DMA drain
      psum_pre[e,c]  = xg[e,c] via identity matmul, 512-wide  (PE, start=True)
      psum_pre[e,c] += attr_aug[e,:17] @ Wbond_aug            (PE, per tile)
      msg = gelu(psum_pre)                                    (ACT, [128,1024] per op)
      shot[e,t,n] = (iota[n]==col_local) * s  (2 batched DVE tensor_tensor ops
        per chunk using free-dim stride-0 broadcast APs of colf/sval)
      psum_accT[c,n] += msg[e,c]^T @ shot[e,n]                (PE, per-block accum)
    Per block flush: accT -> SBUF, fin[c2,n] = Wlin^T @ accT, + b_lin, DMA out.
  - Output is produced transposed ([128, 1250] per core); host concatenates and
    transposes. No collectives needed (disjoint output ranges per core).
  Measured on trn2 (8 cores): ~295 us HW exec, rel err ~2.8e-3 vs f32 reference.
  Engine balance at that point: GPSIMD (gather desc-gen) 248 us, DVE 219 us,
  PE 210 us, ACT 95 us, with the gather stream fully packed (no idle).
"""

import math

import numpy as np
import ml_dtypes

BF16 = ml_dtypes.bfloat16

N_NODES = 10000
IN_C = 128
OUT_C = 128
BOND_F = 16
N_EDGES = 640000
N_CORES = 8
NODES_PER_CORE = N_NODES // N_CORES  # 1250
N_BLOCKS = 10  # per core
BLOCK_NODES = NODES_PER_CORE // N_BLOCKS  # 125
TILE_E = 128
PS_TILES = 8  # tiles per psum/gelu chunk (1024 edges, 2 PSUM banks)
GC_TILES = 32  # tiles per dma_gather chunk (4096 edges)


def _preprocess(x, edge_attr, edge_weight, W_bond, b_bond, W_lin, b_lin, edge_index):
    """Bucket edges by destination, build per-core device arrays."""
    E = edge_index.shape[1]
    n = N_NODES
    row = edge_index[0].astype(np.int64)
    col = edge_index[1].astype(np.int64)
    sl = np.arange(n, dtype=np.int64)
    row_f = np.concatenate([row, sl])
    col_f = np.concatenate([col, sl])
    ew_f = np.concatenate([edge_weight[:, 0].astype(np.float64), np.ones(n)])

    deg_r = np.bincount(row_f, minlength=n).astype(np.float64)
    deg_c = np.bincount(col_f, minlength=n).astype(np.float64)
    inv_r = np.where(deg_r > 0, 1.0 / np.sqrt(np.maximum(deg_r, 1.0)), 0.0)
    inv_c = np.where(deg_c > 0, 1.0 / np.sqrt(np.maximum(deg_c, 1.0)), 0.0)
    s_full = (inv_r[row_f] * inv_c[col_f] * ew_f).astype(np.float32)

    EF = E + n  # full edge count incl self-loops
    # bucket id: 80 buckets of 125 consecutive dest nodes
    bucket = col_f // BLOCK_NODES  # in [0, 80)
    order = np.argsort(bucket, kind="stable")
    bucket_sorted = bucket[order]
    counts = np.bincount(bucket_sorted, minlength=N_CORES * N_BLOCKS)
    T_blk = max(1, int(math.ceil(counts.max() / TILE_E)))
    cap = T_blk * TILE_E
    T_total = N_BLOCKS * T_blk
    E_pad = T_total * TILE_E  # per core

    # slot each edge into its bucket's padded range
    starts = np.zeros(N_CORES * N_BLOCKS, dtype=np.int64)
    starts[1:] = np.cumsum(counts)[:-1]
    within = np.arange(EF) - starts[bucket_sorted]
    glob_slot = bucket_sorted * cap + within  # position in the 80*cap global array

    rows_g = np.zeros(N_CORES * N_BLOCKS * cap, dtype=np.int16)
    colf_g = np.full(N_CORES * N_BLOCKS * cap, -1.0, dtype=np.float32)
    s_g = np.zeros(N_CORES * N_BLOCKS * cap, dtype=np.float32)
    attrT_g = np.zeros((BOND_F + 1, N_CORES * N_BLOCKS * cap), dtype=BF16)

    eids = order  # original edge ids in sorted order
    rows_g[glob_slot] = row_f[eids].astype(np.int16)
    colf_g[glob_slot] = (col_f[eids] % BLOCK_NODES).astype(np.float32)
    s_g[glob_slot] = s_full[eids]
    real = eids < E  # not a self-loop
    rs = glob_slot[real]
    attrT_g[:BOND_F, rs] = edge_attr[eids[real]].T.astype(BF16)
    attrT_g[BOND_F, rs] = 1.0

    # per-core views
    per_core = []
    for c in range(N_CORES):
        lo, hi = c * N_BLOCKS * cap, (c + 1) * N_BLOCKS * cap
        rows_c = rows_g[lo:hi]
        # wrap gather idxs: per gather chunk, position i -> [i % 16, i // 16]
        gidx = np.zeros((16, E_pad // 16), dtype=np.int16)
        for g0 in range(0, T_total, GC_TILES):
            g1 = min(g0 + GC_TILES, T_total)
            seg = rows_c[g0 * TILE_E : g1 * TILE_E]
            cols = seg.shape[0] // 16
            gidx[:, g0 * 8 : g0 * 8 + cols] = seg.reshape(cols, 16).T
        per_core.append(
            dict(
                attrT=np.ascontiguousarray(attrT_g[:, lo:hi]),
                gidx=np.ascontiguousarray(np.tile(gidx, (8, 1))),
                colf=np.ascontiguousarray(colf_g[lo:hi].reshape(T_total, TILE_E).T),
                sval=np.ascontiguousarray(s_g[lo:hi].reshape(T_total, TILE_E).T),
            )
        )

    consts = dict(
        xrows=np.ascontiguousarray(x.astype(BF16)),
        wbond=np.ascontiguousarray(
            np.concatenate([W_bond, b_bond[None, :]], axis=0).astype(BF16)
        ),
        wlin=np.ascontiguousarray(W_lin.astype(BF16)),
        blin=np.ascontiguousarray(b_lin.astype(np.float32).reshape(128, 1)),
        iotam=np.ascontiguousarray(
            np.broadcast_to(
                np.tile(np.arange(128, dtype=np.float32), PS_TILES), (128, PS_TILES * 128)
            )
        ),
        ident=np.eye(128, dtype=BF16),
    )
    return per_core, consts, T_blk


def _build_program(T_blk):
    import concourse.bass as bass
    import concourse.tile as tile
    from concourse import bacc, mybir

    f32 = mybir.dt.float32
    bf16 = mybir.dt.bfloat16
    i16 = mybir.dt.int16
    T_total = N_BLOCKS * T_blk
    E_pad = T_total * TILE_E

    nc = bacc.Bacc("TRN2", target_bir_lowering=False, debug=False, num_swdge_queues=4)

    xrows_d = nc.dram_tensor("xrows", [N_NODES, IN_C], bf16, kind="ExternalInput")
    attrT_d = nc.dram_tensor("attrT", [BOND_F + 1, E_pad], bf16, kind="ExternalInput")
    gidx_d = nc.dram_tensor("gidx", [128, E_pad // 16], i16, kind="ExternalInput")
    colf_d = nc.dram_tensor("colf", [128, T_total], f32, kind="ExternalInput")
    sval_d = nc.dram_tensor("sval", [128, T_total], f32, kind="ExternalInput")
    wbond_d = nc.dram_tensor("wbond", [BOND_F + 1, 128], bf16, kind="ExternalInput")
    wlin_d = nc.dram_tensor("wlin", [128, 128], bf16, kind="ExternalInput")
    blin_d = nc.dram_tensor("blin", [128, 1], f32, kind="ExternalInput")
    iotam_d = nc.dram_tensor("iotam", [128, PS_TILES * 128], f32, kind="ExternalInput")
    ident_d = nc.dram_tensor("ident", [128, 128], bf16, kind="ExternalInput")
    outT_d = nc.dram_tensor(
        "outT", [128, NODES_PER_CORE], f32, kind="ExternalOutput"
    )

    is_equal = mybir.AluOpType.is_equal
    mult = mybir.AluOpType.mult
    GELU = mybir.ActivationFunctionType.Gelu
    IDENT = mybir.ActivationFunctionType.Identity

    with tile.TileContext(nc) as tc:
        with (
            tc.tile_pool(name="const", bufs=1) as constp,
            tc.tile_pool(name="scal", bufs=1) as scalp,
            tc.tile_pool(name="attr", bufs=4) as attrp,
            tc.tile_pool(name="xg", bufs=8) as xgp,
            tc.tile_pool(name="msg", bufs=4) as msgp,
            tc.tile_pool(name="shot", bufs=4) as shotp,
            tc.tile_pool(name="accs", bufs=2) as accsp,
            tc.tile_pool(name="outb", bufs=2) as outbp,
            tc.tile_pool(name="pspre", bufs=2, space="PSUM") as pspre,
            tc.tile_pool(name="psout", bufs=2, space="PSUM") as psout,
            tc.tile_pool(name="psfin", bufs=1, space="PSUM") as psfin,
        ):
            gidx_sb = scalp.tile([128, E_pad // 16], i16)
            first_cols = GC_TILES * 8
            nc.sync.dma_start(gidx_sb[:, :first_cols], gidx_d[:, :first_cols])
            iotam_sb = constp.tile([128, PS_TILES, 128], f32)
            nc.sync.dma_start(iotam_sb[:], iotam_d[:].rearrange("p (t n) -> p t n", n=128))
            ident_sb = constp.tile([128, 128], bf16)
            nc.sync.dma_start(ident_sb[:], ident_d[:])
            wbond_sb = constp.tile([BOND_F + 1, 128], bf16)
            nc.sync.dma_start(wbond_sb[:], wbond_d[:])
            wlin_sb = constp.tile([128, 128], bf16)
            nc.sync.dma_start(wlin_sb[:], wlin_d[:])
            blin_sb = constp.tile([128, 1], f32)
            nc.sync.dma_start(blin_sb[:], blin_d[:])
            colf_sb = scalp.tile([128, T_total], f32)
            nc.sync.dma_start(colf_sb[:], colf_d[:])
            sval_sb = scalp.tile([128, T_total], f32)
            nc.sync.dma_start(sval_sb[:], sval_d[:])
            nc.sync.dma_start(gidx_sb[:, first_cols:], gidx_d[:, first_cols:])

            n_pchunks = (T_total + PS_TILES - 1) // PS_TILES
            # gather schedule: 16-tile chunks, last ~32 tiles tapered to 4-tile
            # chunks so the compute tail overlaps the final gathers
            gather_sizes = []
            rem = T_total
            while rem > 2 * GC_TILES:
                gather_sizes.append(GC_TILES)
                rem -= GC_TILES
            while rem > 0:
                # taper in PS_TILES-aligned steps so each psum chunk stays
                # within a single gather chunk
                take = min(PS_TILES, rem)
                gather_sizes.append(take)
                rem -= take
            cur_xg = None
            cur_g0 = 0
            next_g0 = 0
            gather_idx = 0
            cur_acc = None

            for pc in range(n_pchunks):
                t0 = pc * PS_TILES
                t1 = min(t0 + PS_TILES, T_total)
                nt = t1 - t0

                if t0 >= next_g0:
                    g0 = next_g0
                    gn = gather_sizes[gather_idx]
                    cur_xg = xgp.tile([128, GC_TILES, IN_C], bf16)
                    nc.gpsimd.dma_gather(
                        cur_xg[:, :gn, :],
                        xrows_d[:],
                        gidx_sb[:, g0 * 8 : g0 * 8 + gn * 8],
                        gn * TILE_E,
                        gn * TILE_E,
                        IN_C,
                        single_packet=False,
                        queue_num=gather_idx % 4,
                    )
                    cur_g0 = g0
                    next_g0 = g0 + gn
                    gather_idx += 1

                pre = pspre.tile([128, PS_TILES * 128], f32)
                attr_sb = attrp.tile([BOND_F + 1, PS_TILES * 128], bf16)
                nc.sync.dma_start(
                    attr_sb[:, : nt * 128], attrT_d[:, t0 * 128 : t1 * 128]
                )
                for j0 in range(0, nt, 4):
                    j1 = min(j0 + 4, nt)
                    nc.tensor.matmul(
                        pre[:, j0 * 128 : j1 * 128],
                        ident_sb[:],
                        cur_xg[:, t0 - cur_g0 + j0 : t0 - cur_g0 + j1, :],
                        start=True,
                        stop=False,
                        skip_group_check=True,
                    )
                for j in range(nt):
                    sl = slice(j * 128, (j + 1) * 128)
                    nc.tensor.matmul(
                        pre[:, sl],
                        attr_sb[:, sl],
                        wbond_sb[:],
                        start=False,
                        stop=True,
                        skip_group_check=True,
                    )
                msg = msgp.tile([128, PS_TILES * 128], bf16)
                nc.scalar.activation(msg[:, : nt * 128], pre[:, : nt * 128], GELU)

                shot3 = shotp.tile([128, PS_TILES, 128], bf16)
                nc.vector.tensor_tensor(
                    out=shot3[:, :nt, :],
                    in0=iotam_sb[:, :nt, :],
                    in1=colf_sb[:, t0:t1].to_broadcast([128, nt, 128]),
                    op=is_equal,
                )
                nc.vector.tensor_tensor(
                    out=shot3[:, :nt, :],
                    in0=shot3[:, :nt, :],
                    in1=sval_sb[:, t0:t1].to_broadcast([128, nt, 128]),
                    op=mult,
                )

                for j in range(nt):
                    t = t0 + j
                    b = t // T_blk
                    tin = t % T_blk
                    if tin == 0:
                        cur_acc = psout.tile([128, 128], f32)
                    nc.tensor.matmul(
                        cur_acc[:],
                        msg[:, j * 128 : (j + 1) * 128],
                        shot3[:, j, :],
                        start=(tin == 0),
                        stop=(tin == T_blk - 1),
                        skip_group_check=True,
                    )
                    if tin == T_blk - 1:
                        accT = accsp.tile([128, 128], bf16)
                        nc.vector.tensor_copy(accT[:], cur_acc[:])
                        fin = psfin.tile([128, BLOCK_NODES], f32)
                        nc.tensor.matmul(
                            fin[:],
                            wlin_sb[:],
                            accT[:, :BLOCK_NODES],
                            start=True,
                            stop=True,
                            skip_group_check=True,
                        )
                        outb = outbp.tile([128, BLOCK_NODES], f32)
                        nc.scalar.activation(
                            outb[:], fin[:], IDENT, bias=blin_sb[:, 0:1]
                        )
                        nc.sync.dma_start(
                            outT_d[:, b * BLOCK_NODES : (b + 1) * BLOCK_NODES],
                            outb[:],
                        )

    nc.compile()
    return nc


def _run(inputs, trace=False):
    from concourse.bass_utils import run_bass_kernel_spmd

    per_core, consts, T_blk = _preprocess(**inputs)
    nc = _build_program(T_blk)
    in_maps = [{**consts, **pc} for pc in per_core]
    res = run_bass_kernel_spmd(nc, in_maps, list(range(N_CORES)), trace=trace)
    outT = np.concatenate([res.results[c]["outT"] for c in range(N_CORES)], axis=1)
    out = np.ascontiguousarray(outT.T).astype(np.float32)
    return out, res


def kernel(**inputs):
    out, _ = _run(inputs, trace=False)
    return out
